# revision 1
# baseline (speedup 1.0000x reference)
"""Trainium2 Bass kernel for GNN message passing (nn_MessageModel).

Reference computation:
    inp = concat([x[col], edge_attr], 1)          # [E, 48]
    h = relu(inp @ W1 + b1)                       # [E, 64]
    messages = h @ W2 + b2                        # [E, 32]
    out = segment_sum(messages, row, N)           # [N, 32]

Strategy (8 NeuronCores, SPMD):
- Host: sort edges by destination row; shard by destination node range
  (12500 nodes/core) so per-core outputs are disjoint (no collective
  needed). Build feature-major inp^T tiles (x[col] rows + edge_attr,
  transposed) in sorted edge order, packed two 512-edge "lanes" per
  96-partition rhs tile.
- HW per 2048-edge supergroup (4 lane-packed 512-edge node-aligned
  groups): one DMA loads inp^T [96, 1024]; per pair of lanes one
  block-diagonal W1 matmul -> h_pre PSUM [128, 512]; DVE bias+relu;
  block-diagonal W2 matmul -> msg^T PSUM [128, 512] (4 lanes x 32
  features); DVE cumsum scan along edges; GPSIMD ap_gather extracts
  each node's last-edge cumsum column; DMA out.
- Host: per-node adjacent differences of the extracted cumsums, add
  deg * b2, assemble [N, 32].
"""
import sys

if "/opt/trn_rl_repo" not in sys.path:
    sys.path.insert(0, "/opt/trn_rl_repo")

import numpy as np
import ml_dtypes

BF16 = ml_dtypes.bfloat16

N_NODES = 100000
N_EDGES = 1600000
D_NODE = 32
D_EDGE = 16
D_IN = D_NODE + D_EDGE
D_HID = 64
D_OUT = 32

N_CORES = 8
NPC = N_NODES // N_CORES          # nodes per core
P = 128
GE = 512                          # edges per scan group (lane)
EXT_BATCH = 8                     # supergroups of ext output per out-DMA
LANES = 4                         # lanes per supergroup
SGE = GE * LANES                  # edges per supergroup
NODE_SLOTS = 64                   # max nodes per group
EDGE_CAP = 464                    # grouping edge budget per group
NODE_CAP = 56                     # grouping node budget per group

_compiled_cache = {}


# ----------------------------------------------------------------------------
# host-side preprocessing
# ----------------------------------------------------------------------------

def _preprocess(x, edge_index, edge_attr, W1, b1, W2, b2):
    x = np.asarray(x, dtype=np.float32)
    W1 = np.asarray(W1, dtype=np.float32)
    W2 = np.asarray(W2, dtype=np.float32)
    row = np.asarray(edge_index[0], dtype=np.int64)
    col = np.asarray(edge_index[1], dtype=np.int64)
    order = np.argsort(row, kind="stable")
    col_s = col[order].astype(np.int64)
    attr_s = np.asarray(edge_attr, dtype=np.float32)[order]

    deg = np.bincount(row, minlength=N_NODES).astype(np.int64)
    cum_deg = np.zeros(N_NODES + 1, dtype=np.int64)
    np.cumsum(deg, out=cum_deg[1:])

    cores = []
    max_groups = 0
    for k in range(N_CORES):
        n0, n1 = k * NPC, (k + 1) * NPC
        e0, e1 = cum_deg[n0], cum_deg[n1]
        d_k = deg[n0:n1]
        node_e0 = cum_deg[n0:n1] - e0
        g1 = node_e0 // EDGE_CAP
        g2 = np.arange(NPC) // NODE_CAP
        _, g = np.unique(np.maximum(g1, g2), return_inverse=True)
        n_groups = int(g[-1]) + 1
        grp_node_start = np.searchsorted(g, np.arange(n_groups), side="left")
        grp_node_end = np.searchsorted(g, np.arange(n_groups), side="right")
        grp_edge_start = node_e0[grp_node_start]
        grp_edge_end = np.where(
            grp_node_end < NPC, node_e0.take(grp_node_end, mode="clip"), e1 - e0
        )
        assert (grp_edge_end - grp_edge_start).max() <= GE
        assert (grp_node_end - grp_node_start).max() <= NODE_SLOTS
        cores.append(
            dict(k=k, e0=e0, e1=e1, n_groups=n_groups, d_k=d_k,
                 grp_node_start=grp_node_start, grp_node_end=grp_node_end,
                 grp_edge_start=grp_edge_start, grp_edge_end=grp_edge_end)
        )
        max_groups = max(max_groups, n_groups)

    n_super = -(-max_groups // LANES)

    # inp^T tiles: [core, sg, 96, 1024]; pair a cols a*512.., lane half rows 48*h..
    inpT_in = np.zeros((N_CORES, n_super, 2 * D_IN, 2 * GE), dtype=BF16)
    idx_in = np.zeros((N_CORES, n_super, P, NODE_SLOTS // 16), dtype=np.int16)
    # (flattened later to [core, 128, n_super*4] for the single preload DMA)

    jj = np.arange(NODE_SLOTS)
    for c in cores:
        k = c["k"]
        e0 = c["e0"]
        col_k = col_s[e0:c["e1"]]
        attr_k = attr_s[e0:c["e1"]]
        for gi in range(c["n_groups"]):
            sg, l = divmod(gi, LANES)
            pair, half = divmod(l, 2)
            es, ee = int(c["grp_edge_start"][gi]), int(c["grp_edge_end"][gi])
            cnt = ee - es
            blk = inpT_in[k, sg, half * D_IN:(half + 1) * D_IN,
                          pair * GE:pair * GE + cnt]
            blk[:D_NODE] = x[col_k[es:ee]].T.astype(BF16)
            blk[D_NODE:] = attr_k[es:ee].T.astype(BF16)
            ns, ne = int(c["grp_node_start"][gi]), int(c["grp_node_end"][gi])
            dloc = c["d_k"][ns:ne]
            last = np.maximum(np.cumsum(dloc) - 1, 0).astype(np.int16)
            nn = ne - ns
            j = jj[:nn]
            idx_in[k, sg, 32 * l + (j % 16), j // 16] = last
            idx_in[k, sg, 32 * l + 16 + (j % 16), j // 16] = last

    b1_tile = np.tile(np.asarray(b1, dtype=np.float32)[:, None], (2, 1))  # [128,1]
    W1blk = np.zeros((2 * D_IN, P), dtype=BF16)                            # [96, 128]
    W1blk[:D_IN, :D_HID] = W1
    W1blk[D_IN:, D_HID:] = W1
    W2blk = np.zeros((P, 2 * D_OUT), dtype=BF16)                           # [128, 64]
    W2blk[:D_HID, :D_OUT] = W2
    W2blk[D_HID:, D_OUT:] = W2

    return dict(
        cores=cores, n_super=n_super, inpT_in=inpT_in, idx_in=idx_in,
        b1_tile=b1_tile, W1blk=W1blk, W2blk=W2blk,
        deg=deg, b2=np.asarray(b2, dtype=np.float32),
    )


# ----------------------------------------------------------------------------
# numpy simulation of the HW dataflow (for correctness debugging)
# ----------------------------------------------------------------------------

def _simulate_hw(prep):
    n_super = prep["n_super"]
    W1blk, W2blk = prep["W1blk"], prep["W2blk"]
    b1t = prep["b1_tile"][:, 0]
    ext_all = np.zeros((N_CORES, n_super, P, NODE_SLOTS), dtype=np.float32)
    for k in range(N_CORES):
        for sg in range(n_super):
            inpT = prep["inpT_in"][k, sg].astype(np.float32)
            W1f = W1blk.astype(np.float32)
            W2f = W2blk.astype(np.float32)
            for pair in range(2):
                rhs = inpT[:, pair * GE:(pair + 1) * GE]      # [96, 512]
                h_pre = W1f.T @ rhs                            # [128, 512]
                h = np.maximum(h_pre + b1t[:, None], 0.0).astype(BF16).astype(np.float32)
                msg = W2f.T @ h                                # [64, 512]
                cum = np.cumsum(msg.astype(np.float64), axis=1).astype(np.float32)
                for half in range(2):
                    l = 2 * pair + half
                    idxw = prep["idx_in"][k, sg, 32 * l:32 * l + 16]
                    idx = np.zeros(NODE_SLOTS, dtype=np.int64)
                    for j in range(NODE_SLOTS):
                        idx[j] = idxw[j % 16, j // 16]
                    ext_all[k, sg, 32 * l:32 * l + 32, :] = \
                        cum[32 * half:32 * half + 32][:, idx]
    return ext_all


# ----------------------------------------------------------------------------
# assembly of the final output from extracted cumsums
# ----------------------------------------------------------------------------

def _assemble(prep, ext_all):
    out = np.zeros((N_NODES, D_OUT), dtype=np.float32)
    deg, b2 = prep["deg"], prep["b2"]
    for c in prep["cores"]:
        k = c["k"]
        for gi in range(c["n_groups"]):
            sg, l = divmod(gi, LANES)
            ns, ne = int(c["grp_node_start"][gi]), int(c["grp_node_end"][gi])
            nn = ne - ns
            v = ext_all[k, sg, 32 * l:32 * l + 32, :nn]
            dv = np.empty_like(v)
            dv[:, 0] = v[:, 0]
            dv[:, 1:] = v[:, 1:] - v[:, :-1]
            out[k * NPC + ns:k * NPC + ne] = dv.T
    out[deg == 0] = 0.0
    out += deg[:, None] * b2[None, :]
    return out


# ----------------------------------------------------------------------------
# bass kernel
# ----------------------------------------------------------------------------

def _build_bass(n_super):
    import concourse.bacc as bacc
    import concourse.mybir as mybir
    import concourse.tile as tile
    from concourse.tile_rust import add_dep_helper
    from contextlib import ExitStack

    nc = bacc.Bacc("TRN2", target_bir_lowering=False, debug=False,
                   enable_asserts=True, num_devices=N_CORES)
    f32 = mybir.dt.float32
    bf16 = mybir.dt.bfloat16
    inpT_d = nc.dram_tensor("inpT", [n_super, 2 * D_IN, 2 * GE], bf16, kind="ExternalInput").ap()
    idx_d = nc.dram_tensor("idx", [P, n_super * (NODE_SLOTS // 16)], mybir.dt.int16, kind="ExternalInput").ap()
    W1_d = nc.dram_tensor("W1blk", [2 * D_IN, P], bf16, kind="ExternalInput").ap()
    W2_d = nc.dram_tensor("W2blk", [P, 2 * D_OUT], bf16, kind="ExternalInput").ap()
    b1_d = nc.dram_tensor("b1t", [P, 1], f32, kind="ExternalInput").ap()
    ext_d = nc.dram_tensor("ext", [P, n_super * NODE_SLOTS], f32, kind="ExternalOutput").ap()

    with tile.TileContext(nc) as tc, ExitStack() as ctx:
        const = ctx.enter_context(tc.tile_pool(name="const", bufs=1))
        sb_in = ctx.enter_context(tc.tile_pool(name="sb_in", bufs=6))
        sb_h = ctx.enter_context(tc.tile_pool(name="sb_h", bufs=6))
        sb_out = ctx.enter_context(tc.tile_pool(name="sb_out", bufs=6))
        ps_h = ctx.enter_context(tc.tile_pool(name="ps_h", bufs=4, space="PSUM"))
        ps_m = ctx.enter_context(tc.tile_pool(name="ps_m", bufs=4, space="PSUM"))

        idx_all = const.tile([P, n_super * (NODE_SLOTS // 16)], mybir.dt.int16)
        nc.sync.dma_start(idx_all[:], idx_d[:])
        ones = const.tile([P, GE], f32)
        nc.gpsimd.memset(ones[:], 1.0)
        W1_s = const.tile([2 * D_IN, P], bf16)
        nc.sync.dma_start(W1_s[:], W1_d[:])
        W2_s = const.tile([P, 2 * D_OUT], bf16)
        nc.sync.dma_start(W2_s[:], W2_d[:])
        b1_s = const.tile([P, 1], f32)
        nc.sync.dma_start(b1_s[:], b1_d[:])

        ext_tiles = []
        pe_chain = []

        def chain(inst):
            if pe_chain:
                add_dep_helper(inst.ins, pe_chain[-1].ins, sync=False,
                               reason="PE weight-batch order")
            pe_chain.append(inst)

        BATCH = 1
        batches = [list(range(b, min(b + BATCH, n_super)))
                   for b in range(0, n_super, BATCH)]

        inps, msgs, hpres, hss = {}, {}, {}, {}

        def emit_w1(bi):
            for sg in batches[bi]:
                inpT_s = sb_in.tile([2 * D_IN, 2 * GE], bf16, tag="inpT",
                                    name=f"inp{sg}")
                nc.sync.dma_start(inpT_s[:], inpT_d[sg])
                inps[sg] = inpT_s
            for sg in batches[bi]:
                for pair in range(2):
                    h_pre = ps_h.tile([P, GE], f32, tag="hpre",
                                      name=f"hp{sg}_{pair}")
                    mm = nc.tensor.matmul(
                        h_pre[:], lhsT=W1_s[:],
                        rhs=inps[sg][:, pair * GE:(pair + 1) * GE],
                        start=True, stop=True,
                    )
                    chain(mm)
                    hpres[(sg, pair)] = h_pre
            for sg in batches[bi]:
                for pair in range(2):
                    h_s = sb_h.tile([P, GE], bf16, tag="hs",
                                    name=f"hs{sg}_{pair}")
                    nc.scalar.activation(
                        out=h_s[:], in_=hpres[(sg, pair)][:],
                        func=mybir.ActivationFunctionType.Relu, bias=b1_s[:],
                    )
                    hss[(sg, pair)] = h_s

        def emit_w2(bi):
            for sg in batches[bi]:
                msg_p = ps_m.tile([P, GE], f32, tag="msg", name=f"mp{sg}")
                for pair in range(2):
                    mm = nc.tensor.matmul(
                        msg_p[64 * pair:64 * pair + 64, :], lhsT=W2_s[:],
                        rhs=hss[(sg, pair)][:],
                        start=True, stop=True,
                    )
                    chain(mm)
                msgs[sg] = msg_p
            for sg in batches[bi]:
                cum_s = sb_out.tile([P, GE], f32, tag="cum", name=f"cum{sg}")
                nc.vector.tensor_tensor_scan(
                    out=cum_s[:], data0=ones[:], data1=msgs[sg][:], initial=0.0,
                    op0=mybir.AluOpType.mult, op1=mybir.AluOpType.add,
                )
                bi2 = sg % EXT_BATCH
                if bi2 == 0:
                    ext_s = sb_out.tile([P, EXT_BATCH * NODE_SLOTS], f32,
                                        tag="ext", name=f"ext{sg}")
                    ext_tiles.append(ext_s)
                ext_s = ext_tiles[-1]
                nc.gpsimd.ap_gather(
                    out_ap=ext_s[:, bi2 * NODE_SLOTS:(bi2 + 1) * NODE_SLOTS],
                    in_ap=cum_s[:],
                    idxs_ap=idx_all[:, sg * (NODE_SLOTS // 16):(sg + 1) * (NODE_SLOTS // 16)],
                    channels=P, num_elems=GE, d=1, num_idxs=NODE_SLOTS,
                )
                if bi2 == EXT_BATCH - 1 or sg == n_super - 1:
                    b0 = sg - bi2
                    nc.sync.dma_start(
                        ext_d[:, b0 * NODE_SLOTS:(sg + 1) * NODE_SLOTS],
                        ext_s[:, :(bi2 + 1) * NODE_SLOTS],
                    )

        LAG = 2
        for bi in range(len(batches)):
            emit_w1(bi)
            if bi >= LAG:
                emit_w2(bi - LAG)
        for bi in range(max(0, len(batches) - LAG), len(batches)):
            emit_w2(bi)

    nc.compile()
    return nc


def _run_hw(prep, trace=False):
    from concourse.bass_utils import run_bass_kernel_spmd

    n_super = prep["n_super"]
    if n_super not in _compiled_cache:
        _compiled_cache[n_super] = _build_bass(n_super)
    nc = _compiled_cache[n_super]

    in_maps = []
    for k in range(N_CORES):
        idx_flat = prep["idx_in"][k].transpose(1, 0, 2).reshape(P, -1)
        in_maps.append({
            "inpT": prep["inpT_in"][k],
            "idx": idx_flat,
            "W1blk": prep["W1blk"],
            "W2blk": prep["W2blk"],
            "b1t": prep["b1_tile"],
        })
    res = run_bass_kernel_spmd(nc, in_maps, list(range(N_CORES)), trace=trace)
    ext_all = np.stack([
        res.results[k]["ext"].reshape(P, n_super, NODE_SLOTS).transpose(1, 0, 2)
        for k in range(N_CORES)
    ])
    return ext_all, res


def kernel(x, edge_index, edge_attr, W1, b1, W2, b2, _numpy_sim=False):
    prep = _preprocess(x, edge_index, edge_attr, W1, b1, W2, b2)
    if _numpy_sim:
        ext_all = _simulate_hw(prep)
    else:
        ext_all, _ = _run_hw(prep)
    return _assemble(prep, ext_all)



# revision 3
# speedup vs baseline: 1.9988x; 1.9988x over previous
"""Trainium2 Bass kernel for GNN message passing (nn_MessageModel).

Reference computation:
    inp = concat([x[col], edge_attr], 1)          # [E, 48]
    h = relu(inp @ W1 + b1)                       # [E, 64]
    messages = h @ W2 + b2                        # [E, 32]
    out = segment_sum(messages, row, N)           # [N, 32]

Strategy (8 NeuronCores, SPMD, destination-node sharding):
- Host: split high-degree nodes into virtual nodes (deg <= CAP), sort
  all virtual nodes by degree (desc), deal them round-robin so each
  core gets an identical degree profile. Nodes are grouped into
  512-slot blocks; block pairs share matmul tiles (2 lanes packed via
  block-diagonal weights). Each (pair, round) is one [96, 512] rhs
  tile holding the r-th edge of each node slot (zero for pad slots).
- HW per (pair, round): W1 matmul [96 -> 128] -> psum; bias+relu
  (alternating Scalar/DVE) -> h bf16; W2 matmul [128 -> 64]
  accumulating rounds into a per-pair PSUM accumulator (start on
  round 0). Segment-sum thus happens in PSUM for free. Drain each
  pair's accumulator to SBUF + DMA out.
- Host: scatter accumulator columns back to node ids, subtract the
  deterministic pad contribution relu(b1) @ W2 per padded round, add
  deg * b2, and merge virtual-node partials.
"""
import sys

if "/opt/trn_rl_repo" not in sys.path:
    sys.path.insert(0, "/opt/trn_rl_repo")

import numpy as np
import ml_dtypes

BF16 = ml_dtypes.bfloat16

N_NODES = 100000
N_EDGES = 1600000
D_NODE = 32
D_EDGE = 16
D_IN = D_NODE + D_EDGE
D_HID = 64
D_OUT = 32

N_CORES = 8
P = 128
GE = 512                 # node slots per block (psum bank cols, f32)
CAP = 24                 # max edges per virtual node (accum chain bound)
B = 4                    # units per input DMA batch
CHUNK = 8                # W1/W2 PE interleave granularity

_compiled_cache = {}


# ----------------------------------------------------------------------------
# host-side preprocessing
# ----------------------------------------------------------------------------

def _preprocess(x, edge_index, edge_attr, W1, b1, W2, b2):
    x = np.asarray(x, dtype=np.float32)
    W1 = np.asarray(W1, dtype=np.float32)
    W2 = np.asarray(W2, dtype=np.float32)
    b1 = np.asarray(b1, dtype=np.float32)
    b2 = np.asarray(b2, dtype=np.float32)
    row = np.asarray(edge_index[0], dtype=np.int64)
    col = np.asarray(edge_index[1], dtype=np.int64)
    E = row.shape[0]

    order = np.argsort(row, kind="stable")
    col_s = col[order]
    attr_s = np.asarray(edge_attr, dtype=np.float32)[order]
    erow = row[order]

    deg = np.bincount(row, minlength=N_NODES).astype(np.int64)
    cum = np.zeros(N_NODES + 1, dtype=np.int64)
    np.cumsum(deg, out=cum[1:])

    # virtual nodes: chunks of CAP edges
    nv_per = np.maximum(1, -(-deg // CAP))
    vbase = np.zeros(N_NODES + 1, dtype=np.int64)
    np.cumsum(nv_per, out=vbase[1:])
    NV0 = int(vbase[-1])
    vnode_node = np.repeat(np.arange(N_NODES), nv_per)
    vi = np.arange(NV0) - np.repeat(vbase[:-1], nv_per)
    vdeg = np.minimum(deg[vnode_node] - vi * CAP, CAP)

    # sort virtual nodes by degree desc
    vorder = np.argsort(-vdeg, kind="stable")       # vrank -> vnode
    vrank_of = np.empty(NV0, dtype=np.int64)
    vrank_of[vorder] = np.arange(NV0)
    SB = 4096 * 2                                    # ranks per block pair
    NVpad = -(-NV0 // SB) * SB
    vdeg_sorted = np.zeros(NVpad, dtype=np.int64)
    vdeg_sorted[:NV0] = vdeg[vorder]

    npairs_all = NVpad // SB
    R = vdeg_sorted[np.arange(npairs_all) * SB].astype(np.int64)
    npairs = int(np.sum(R > 0))
    R = R[:npairs]
    assert np.all(R[:-1] >= R[1:]) if npairs > 1 else True
    unit_base = np.zeros(npairs + 1, dtype=np.int64)
    np.cumsum(R, out=unit_base[1:])
    U = int(unit_base[-1])
    NB = -(-U // B)

    # per-edge placement
    epos = np.arange(E) - cum[erow]
    ev_i = epos // CAP
    r_e = epos - ev_i * CAP
    evr = vrank_of[vbase[erow] + ev_i]
    j_e = evr // 4096
    w_e = evr % 4096
    s_e = w_e // 8
    k_e = w_e % 8
    lane_e = j_e % 2
    pair_e = j_e // 2
    u_e = unit_base[pair_e] + r_e
    assert np.all(r_e < R[pair_e])

    feats = np.empty((E, D_IN), dtype=BF16)
    feats[:, :D_NODE] = x[col_s]
    feats[:, D_NODE:] = attr_s

    inpT = np.zeros((N_CORES, NB, 2 * D_IN, B * GE), dtype=BF16)
    nb_e = u_e // B
    colpos = (u_e % B) * GE + s_e
    base = ((k_e * NB + nb_e) * (2 * D_IN) + lane_e * D_IN) * (B * GE) + colpos
    idx = base[:, None] + (np.arange(D_IN) * (B * GE))[None, :]
    inpT.reshape(-1)[idx] = feats

    b1_tile = np.tile(b1[:, None], (2, 1))                    # [128, 1]
    W1blk = np.zeros((2 * D_IN, P), dtype=BF16)               # [96, 128]
    W1blk[:D_IN, :D_HID] = W1
    W1blk[D_IN:, D_HID:] = W1
    W2blk = np.zeros((P, 2 * D_OUT), dtype=BF16)              # [128, 64]
    W2blk[:D_HID, :D_OUT] = W2
    W2blk[D_HID:, D_OUT:] = W2

    # pad contribution per padded round: relu(b1) as bf16 through W2
    hpad = np.maximum(b1, 0.0).astype(BF16).astype(np.float32)
    corr = hpad @ W2.astype(BF16).astype(np.float32)          # [32]

    return dict(
        R=R, npairs=npairs, NB=NB, U=U, inpT=inpT,
        b1_tile=b1_tile, W1blk=W1blk, W2blk=W2blk,
        deg=deg, b2=b2, corr=corr,
        vorder=vorder, vdeg=vdeg, vnode_node=vnode_node, NV0=NV0,
    )


# ----------------------------------------------------------------------------
# numpy simulation of the HW dataflow (for correctness debugging)
# ----------------------------------------------------------------------------

def _simulate_hw(prep):
    R, npairs, NB = prep["R"], prep["npairs"], prep["NB"]
    W1f = prep["W1blk"].astype(np.float32)
    W2f = prep["W2blk"].astype(np.float32)
    b1t = prep["b1_tile"][:, 0]
    ext = np.zeros((N_CORES, npairs, 2 * D_OUT, GE), dtype=np.float32)
    for k in range(N_CORES):
        flat = (prep["inpT"][k].astype(np.float32)
                .reshape(NB, 2 * D_IN, B, GE).transpose(0, 2, 1, 3)
                .reshape(NB * B, 2 * D_IN, GE))
        u0 = 0
        for p in range(npairs):
            acc = np.zeros((2 * D_OUT, GE), dtype=np.float32)
            for r in range(R[p]):
                rhs = flat[u0 + r].T                  # [GE, 96] -> use [96, GE]
                hpre = W1f.T @ flat[u0 + r]           # [128, GE]
                h = np.maximum(hpre + b1t[:, None], 0.0).astype(BF16).astype(np.float32)
                acc += W2f.T @ h                       # [64, GE]
            ext[k, p] = acc
            u0 += R[p]
    return ext


# ----------------------------------------------------------------------------
# assembly of the final output
# ----------------------------------------------------------------------------

def _assemble(prep, ext):
    R, npairs = prep["R"], prep["npairs"]
    deg, b2, corr = prep["deg"], prep["b2"], prep["corr"]
    vorder, vdeg, vnode_node, NV0 = (
        prep["vorder"], prep["vdeg"], prep["vnode_node"], prep["NV0"])

    vr = np.arange(NV0)
    vn = vorder                                  # vrank -> vnode idx
    j = vr // 4096
    w = vr % 4096
    s = w // 8
    k = w % 8
    lane = j % 2
    pair = j // 2
    live = pair < npairs

    vals = np.zeros((NV0, D_OUT), dtype=np.float32)
    lv = np.nonzero(live)[0]
    # gather [32] vector for each live vnode
    vals[lv] = ext[k[lv], pair[lv], :, s[lv]].reshape(len(lv), 2, D_OUT)[
        np.arange(len(lv)), lane[lv]]
    npad = np.zeros(NV0, dtype=np.int64)
    npad[lv] = R[pair[lv]] - vdeg[vorder][lv]
    vals -= npad[:, None] * corr[None, :]

    out = np.zeros((N_NODES, D_OUT), dtype=np.float32)
    node_of_vrank = vnode_node[vn]
    np.add.at(out, node_of_vrank, vals)
    out += deg[:, None] * b2[None, :]
    return out


# ----------------------------------------------------------------------------
# bass kernel
# ----------------------------------------------------------------------------

def _build_bass(R, NB):
    import concourse.bacc as bacc
    import concourse.mybir as mybir
    import concourse.tile as tile
    from concourse.tile_rust import add_dep_helper
    from contextlib import ExitStack

    R = list(R)
    npairs = len(R)
    U = sum(R)

    nc = bacc.Bacc("TRN2", target_bir_lowering=False, debug=False,
                   enable_asserts=True, num_devices=N_CORES)
    f32 = mybir.dt.float32
    bf16 = mybir.dt.bfloat16
    inpT_d = nc.dram_tensor("inpT", [NB, 2 * D_IN, B * GE], bf16,
                            kind="ExternalInput").ap()
    W1_d = nc.dram_tensor("W1blk", [2 * D_IN, P], bf16, kind="ExternalInput").ap()
    W2_d = nc.dram_tensor("W2blk", [P, 2 * D_OUT], bf16, kind="ExternalInput").ap()
    b1_d = nc.dram_tensor("b1t", [P, 1], f32, kind="ExternalInput").ap()
    ext_d = nc.dram_tensor("ext", [npairs, 2 * D_OUT, GE], f32,
                           kind="ExternalOutput").ap()

    with tile.TileContext(nc) as tc, ExitStack() as ctx:
        const = ctx.enter_context(tc.tile_pool(name="const", bufs=1))
        sb_in = ctx.enter_context(tc.tile_pool(name="sb_in", bufs=6))
        sb_h = ctx.enter_context(tc.tile_pool(name="sb_h", bufs=56))
        sb_out = ctx.enter_context(tc.tile_pool(name="sb_out", bufs=3))
        ps_h = ctx.enter_context(tc.tile_pool(name="ps_h", bufs=5, space="PSUM"))
        ps_acc = ctx.enter_context(tc.tile_pool(name="ps_acc", bufs=2, space="PSUM"))

        W1_s = const.tile([2 * D_IN, P], bf16)
        nc.sync.dma_start(W1_s[:], W1_d[:])
        W2_s = const.tile([P, 2 * D_OUT], bf16)
        nc.sync.dma_start(W2_s[:], W2_d[:])
        b1_s = const.tile([P, 1], f32)
        nc.sync.dma_start(b1_s[:], b1_d[:])

        pe_chain = []

        def chain(inst):
            if pe_chain:
                add_dep_helper(inst.ins, pe_chain[-1].ins, sync=False,
                               reason="PE order")
            pe_chain.append(inst)

        in_tiles = {}      # batch -> tile
        h_tiles = {}       # unit -> tile
        acc_tiles = {}     # pair -> psum tile
        unit_base = [0]
        for r in R:
            unit_base.append(unit_base[-1] + r)

        relu_cnt = [0]

        def emit_w1(p, r):
            u = unit_base[p] + r
            nb, off = divmod(u, B)
            if nb not in in_tiles:
                t = sb_in.tile([2 * D_IN, B * GE], bf16, tag="inp",
                               name=f"in{nb}")
                nc.sync.dma_start(t[:], inpT_d[nb])
                in_tiles[nb] = t
            hp = ps_h.tile([P, GE], f32, tag="hpre", name=f"hp{u}")
            mm = nc.tensor.matmul(
                hp[:], lhsT=W1_s[:],
                rhs=in_tiles[nb][:, off * GE:(off + 1) * GE],
                start=True, stop=True,
            )
            chain(mm)
            h = sb_h.tile([P, GE], bf16, tag="h", name=f"h{u}")
            if relu_cnt[0] % 2 == 0:
                nc.scalar.activation(
                    out=h[:], in_=hp[:],
                    func=mybir.ActivationFunctionType.Relu, bias=b1_s[:],
                )
            else:
                nc.vector.tensor_scalar(
                    out=h[:], in0=hp[:], scalar1=b1_s[:], scalar2=0.0,
                    op0=mybir.AluOpType.add, op1=mybir.AluOpType.max,
                )
            relu_cnt[0] += 1
            h_tiles[u] = h

        def emit_w2(p, r):
            u = unit_base[p] + r
            if p not in acc_tiles:
                acc_tiles[p] = ps_acc.tile([2 * D_OUT, GE], f32, tag="acc",
                                           name=f"acc{p}")
            mm = nc.tensor.matmul(
                acc_tiles[p][:], lhsT=W2_s[:], rhs=h_tiles[u][:],
                start=(r == 0), stop=(r == R[p] - 1),
                skip_group_check=True,
            )
            chain(mm)
            del h_tiles[u]

        def emit_drain(p):
            o = sb_out.tile([2 * D_OUT, GE], f32, tag="ext", name=f"ext{p}")
            nc.scalar.copy(out=o[:], in_=acc_tiles[p][:])
            nc.sync.dma_start(ext_d[p], o[:])
            del acc_tiles[p]

        # interleave W1 of pair p with W2 of pair p-1 at CHUNK granularity
        w2c = [0] * npairs

        def w2_budget(p, n):
            n = min(n, R[p] - w2c[p])
            for i in range(n):
                emit_w2(p, w2c[p])
                w2c[p] += 1
            if w2c[p] == R[p]:
                emit_drain(p)
                return True
            return False

        for p in range(npairs):
            for c0 in range(0, R[p], CHUNK):
                for r in range(c0, min(c0 + CHUNK, R[p])):
                    emit_w1(p, r)
                if p > 0 and w2c[p - 1] < R[p - 1]:
                    w2_budget(p - 1, CHUNK)
            if p > 0 and w2c[p - 1] < R[p - 1]:
                w2_budget(p - 1, R[p - 1])
        w2_budget(npairs - 1, R[npairs - 1])

    nc.compile()
    return nc


def _run_hw(prep, trace=False):
    from concourse.bass_utils import run_bass_kernel_spmd

    key = (tuple(prep["R"]), prep["NB"])
    if key not in _compiled_cache:
        _compiled_cache[key] = _build_bass(prep["R"], prep["NB"])
    nc = _compiled_cache[key]

    in_maps = []
    for k in range(N_CORES):
        in_maps.append({
            "inpT": prep["inpT"][k],
            "W1blk": prep["W1blk"],
            "W2blk": prep["W2blk"],
            "b1t": prep["b1_tile"],
        })
    res = run_bass_kernel_spmd(nc, in_maps, list(range(N_CORES)), trace=trace)
    ext = np.stack([res.results[k]["ext"] for k in range(N_CORES)])
    return ext, res


def kernel(x, edge_index, edge_attr, W1, b1, W2, b2, _numpy_sim=False):
    prep = _preprocess(x, edge_index, edge_attr, W1, b1, W2, b2)
    if _numpy_sim:
        ext = _simulate_hw(prep)
    else:
        ext, _ = _run_hw(prep)
    return _assemble(prep, ext)
